# revision 23
# baseline (speedup 1.0000x reference)
"""CRF negative-log-likelihood loss kernel for Trainium2 (Bass/Tile).

Problem: B=32, T=512, L=64 linear-chain CRF loss
    loss = sum_b [ -path_score(b) + logZ(b) ]

Algorithm (per core; data-parallel over batch, 4 rows/core):
  logZ via the linear-space scaled forward recurrence, run CONCURRENTLY
  from both ends (forward-backward identity) to halve the serial span:
      F_t[j] = exp(h_t[j]) / S_t          (softmax of emissions, sum=1)
      alpha_t = diag(F_t) E^T alpha_{t-1},   E = exp(trans)
      beta_{s-1} = E (F_s ⊙ beta_s)
      Z = sum_j alpha_m[j] beta_m[j];  logZ = ln Z + sum_t ln S_t
  The F normalization keeps both states bounded (empirically [1,10]) so
  bf16/fp32 stay in range with no max-subtraction (inputs ~ N(0,1)).
  Each chain step: one bf16 PE matmul (stationary E / E^T) + one DVE
  scalar_tensor_tensor (PSUM * F -> SBUF bf16). State is (64 part, 4 b).

  path scores via PSUM-accumulated cross-products (summed over b - the
  loss sums b anyway):
      h_total = trace(inp_flat^T @ lab_flat)
      g_total = <trans, C>,  C = lab_flat[:-1]^T @ lab_flat[1:]
  (boundary tiles use 127 rows so no cross-batch transitions leak in).
  These 32 (128,64,64) matmuls + input prep (exp on ScalarE, transpose
  on PE) are sprinkled between chain steps to hide in engine slack.

Each core emits its partial loss scalar; the host sums the 8 partials
(the scalar all-reduce of the sharding hint).
"""

import functools

import numpy as np

B, T, L = 32, 512, 64
NCORES = 8
BL = B // NCORES  # 4 batch rows per core
P = 128


def build_crf_bass(t_len: int = T):
    """Build the per-core Bass/Tile program. Returns the compiled Bass object."""
    import concourse.bass as bass
    import concourse.bacc as bacc
    import concourse.mybir as mybir
    from concourse import masks
    from concourse import tile

    f32 = mybir.dt.float32
    bf16 = mybir.dt.bfloat16
    AX = mybir.AxisListType
    OP = mybir.AluOpType
    AF = mybir.ActivationFunctionType

    nt = BL * t_len // P  # input tiles
    tpb = t_len // P      # tiles per batch row
    assert t_len % P == 0 and t_len >= 2 * P

    nc = bacc.Bacc("TRN2", target_bir_lowering=False, debug=False,
                   enable_asserts=False)

    inputs = nc.dram_tensor("inputs", [BL, t_len, L], f32, kind="ExternalInput")
    labels = nc.dram_tensor("labels", [BL, t_len, L], f32, kind="ExternalInput")
    trans = nc.dram_tensor("trans", [L, L], f32, kind="ExternalInput")
    out = nc.dram_tensor("out", [1, 1], f32, kind="ExternalOutput")

    inp_flat = inputs.ap().rearrange("b t l -> (b t) l")  # (BL*T, L)
    lab_flat = labels.ap().rearrange("b t l -> (b t) l")

    m = t_len // 2 - 1          # meeting point: alpha_m (x) beta_m
    n_steps = m                 # fwd steps t=1..m ; bwd steps s=T-2..m+1

    with tile.TileContext(nc) as tc:
        with (
            tc.tile_pool(name="const", bufs=1) as const,
            tc.tile_pool(name="stream", bufs=3) as stream,
            tc.tile_pool(name="pst", bufs=2, space=bass.MemorySpace.PSUM) as pst,
            tc.tile_pool(name="psc", bufs=1, space=bass.MemorySpace.PSUM) as psc,
        ):
            ident = const.tile([P, P], f32, tag="ident")
            masks.make_identity(nc, ident[:])
            zeros128 = const.tile([P, 1], f32, tag="z128")
            nc.vector.memset(zeros128[:], 0.0)
            zero1 = const.tile([1, 1], f32, tag="z1")
            nc.vector.memset(zero1[:], 0.0)
            ones128 = const.tile([P, 1], f32, tag="o128")
            nc.vector.memset(ones128[:], 1.0)

            S = const.tile([P, nt], f32, tag="S")      # sum_j exp(h)
            R = const.tile([P, nt], f32, tag="R")      # 1/S
            LS = const.tile([P, nt], f32, tag="LS")    # ln S

            F_rec = const.tile([L, t_len * BL], f32, tag="F_rec")  # F[j, t*4+b]
            tr_sb = const.tile([L, L], f32, tag="tr")
            E_sb = const.tile([L, L], bf16, tag="E")    # exp(trans), lhsT fwd
            E_T = const.tile([L, L], bf16, tag="ET")    # exp(trans)^T, lhsT bwd

            # warm the ACT exp table while the first DMAs are in flight
            tbl = const.tile([1, 1], f32, tag="tbl")
            nc.scalar.activation(tbl[:], zero1[:], AF.Exp, bias=zero1[:])

            nc.sync.dma_start(tr_sb[:], trans.ap())
            nc.scalar.activation(E_sb[:], tr_sb[:], AF.Exp,
                                 bias=zeros128[:L, :])
            psE = pst.tile([L, L], f32, tag="tp")
            nc.tensor.transpose(psE[:], tr_sb[:], ident[:L, :L])
            nc.scalar.activation(E_T[:], psE[:], AF.Exp, bias=zeros128[:L, :])

            # PSUM accumulators for the path scores
            C_ps = psc.tile([L, L], f32, tag="C")       # transition counts
            H_ps = psc.tile([L, L], f32, tag="H")       # inp^T @ lab

            F_v = F_rec[:].rearrange("j (t b) -> j t b", b=BL)

            def f_tile_body(c, head, inp_ap):
                """exp/normalize/transpose one (128,64) input tile into F_rec."""
                b, th = divmod(c, tpb)
                fe_c = stream.tile([P, L], f32, tag="fe")
                nc.scalar.activation(fe_c[:], inp_ap, AF.Exp,
                                     bias=zeros128[:, :1],
                                     accum_out=S[:, c:c + 1])
                nc.vector.reciprocal(R[:, c:c + 1], S[:, c:c + 1])
                fn_c = stream.tile([P, L], f32, tag="fn")
                if head:
                    nc.vector.tensor_scalar_mul(fn_c[:], fe_c[:], R[:, c:c + 1])
                else:
                    nc.scalar.mul(fn_c[:], fe_c[:], R[:, c:c + 1])
                psF = pst.tile([L, P], f32, tag="tp")
                nc.tensor.transpose(psF[:], fn_c[:], ident[:])
                if head:
                    nc.vector.tensor_copy(F_v[:, th * P:(th + 1) * P, b], psF[:])
                else:
                    nc.scalar.copy(F_v[:, th * P:(th + 1) * P, b], psF[:])

            def f_tile(c, head):
                inp_c = stream.tile([P, L], f32, tag="inp")
                nc.sync.dma_start(inp_c[:], inp_flat[c * P:(c + 1) * P, :])
                f_tile_body(c, head, inp_c[:])

            cc_state = {"c": 0, "h": 0}

            def lab_tile(c):
                """Accumulate this tile into the C and H cross-products."""
                kk = P - 1 if (c % tpb == tpb - 1) else P
                inp_c = stream.tile([P, L], f32, tag="inp2")
                nc.sync.dma_start(inp_c[:], inp_flat[c * P:(c + 1) * P, :])
                lab_c = stream.tile([P, L], f32, tag="lab")
                nc.sync.dma_start(lab_c[:], lab_flat[c * P:(c + 1) * P, :])
                lsh_c = stream.tile([P, L], f32, tag="lsh")
                nc.sync.dma_start(lsh_c[:kk, :],
                                  lab_flat[c * P + 1:c * P + 1 + kk, :])
                nc.tensor.matmul(C_ps[:], lab_c[:kk, :], lsh_c[:kk, :],
                                 start=(cc_state["c"] == 0),
                                 stop=(cc_state["c"] == nt - 1),
                                 skip_group_check=True)
                cc_state["c"] += 1
                nc.tensor.matmul(H_ps[:], inp_c[:], lab_c[:],
                                 start=(cc_state["h"] == 0),
                                 stop=(cc_state["h"] == nt - 1),
                                 skip_group_check=True)
                cc_state["h"] += 1

            # F tiles needed to start both chains: th=0 (fwd) and th=tpb-1 (bwd)
            # loaded as one wide DMA per th group, processed with batched
            # (128, BL*64) ops so the head critical path stays short.
            head = [b * tpb for b in range(BL)]
            if tpb > 1:
                head += [b * tpb + tpb - 1 for b in range(BL)]
            inp_4d = inputs.ap().rearrange("b (th p) l -> p th b l", p=P)

            def head_group(th_g):
                big = stream.tile([P, BL * L], f32, tag=f"big{th_g}")
                nc.sync.dma_start(
                    big[:].rearrange("p (b l) -> p b l", b=BL),
                    inp_4d[:, th_g, :, :])
                fe_g = stream.tile([P, BL * L], f32, tag=f"feg{th_g}")
                nc.scalar.activation(fe_g[:], big[:], AF.Exp,
                                     bias=zeros128[:, :1])
                s_g = S[:, th_g::tpb]            # (128, BL) strided
                nc.vector.tensor_reduce(
                    s_g, fe_g[:].rearrange("p (b l) -> p b l", b=BL),
                    axis=AX.X, op=OP.add)
                nc.vector.reciprocal(R[:, th_g::tpb], s_g)
                fn_g = stream.tile([P, BL * L], f32, tag=f"fng{th_g}")
                nc.vector.tensor_mul(
                    fn_g[:].rearrange("p (b l) -> p b l", b=BL),
                    fe_g[:].rearrange("p (b l) -> p b l", b=BL),
                    R[:, th_g::tpb].broadcast_to([P, BL, L]))
                for b in range(BL):
                    fn_c = fn_g[:, b * L:(b + 1) * L]
                    psF = pst.tile([L, P], f32, tag="tp")
                    nc.tensor.transpose(psF[:], fn_c, ident[:])
                    if b % 2 == 0:
                        nc.vector.tensor_copy(
                            F_v[:, th_g * P:(th_g + 1) * P, b], psF[:])
                    else:
                        nc.scalar.copy(
                            F_v[:, th_g * P:(th_g + 1) * P, b], psF[:])

            mid = [c for c in range(nt) if c not in head]

            # sprinkle schedule: remaining F tiles early, label tiles after
            sprinkle = {}
            for i, c in enumerate(mid):
                sprinkle.setdefault(6 + 10 * i, []).append(("F", c))
            lab_start = 6 + 10 * len(mid) + 6
            for i in range(nt):
                sprinkle.setdefault(lab_start + 9 * i, []).append(("L", i))

            alphaF = const.tile([L, BL], bf16, tag="alphaF")
            vB = const.tile([L, BL], bf16, tag="vB")
            head_group(0)
            nc.vector.tensor_copy(alphaF[:], F_rec[:, 0:BL])
            if tpb > 1:
                head_group(tpb - 1)
            nc.vector.tensor_copy(vB[:], F_rec[:, (t_len - 1) * BL:t_len * BL])

            ln_wave = 6 + 10 * len(mid) + 4
            gh_wave = lab_start + 9 * nt + 6
            gh = const.tile([L, 2], f32, tag="gh")

            def emit_ln():
                nc.scalar.activation(LS[:], S[:], AF.Ln, bias=zeros128[:, :1])

            def emit_gh():
                # path scores from the accumulated cross-products:
                #   g_total = <trans, C>;  h_total = trace(H) = <ident, H>
                gtmp = stream.tile([L, L], f32, tag="gtmp")
                nc.vector.scalar_tensor_tensor(gtmp[:], C_ps[:], 1.0, tr_sb[:],
                                               op0=OP.mult, op1=OP.mult)
                nc.vector.tensor_reduce(gh[:, 0:1], gtmp[:], axis=AX.X,
                                        op=OP.add)
                htmp = stream.tile([L, L], f32, tag="htmp")
                nc.vector.scalar_tensor_tensor(htmp[:], H_ps[:], 1.0,
                                               ident[:L, :L],
                                               op0=OP.mult, op1=OP.mult)
                nc.vector.tensor_reduce(gh[:, 1:2], htmp[:], axis=AX.X,
                                        op=OP.add)

            done = {"ln": False, "gh": False}
            with tc.tile_pool(name="psm", bufs=2,
                              space=bass.MemorySpace.PSUM) as psm:
                for k in range(n_steps):
                    for item in sprinkle.get(k, []):
                        if item[0] == "F":
                            f_tile(item[1], head=False)
                        else:
                            lab_tile(item[1])
                    if k == ln_wave:
                        emit_ln()
                        done["ln"] = True
                    if k == gh_wave:
                        emit_gh()
                        done["gh"] = True
                    t = k + 1
                    pf = psm.tile([L, BL], f32, tag="pf")
                    nc.tensor.matmul(pf[:], E_sb[:], alphaF[:],
                                     start=True, stop=True)
                    nc.vector.scalar_tensor_tensor(
                        alphaF[:], pf[:], 1.0,
                        F_rec[:, BL * t:BL * t + BL],
                        op0=OP.mult, op1=OP.mult)
                    s = t_len - 2 - k
                    pb = psm.tile([L, BL], f32, tag="pb")
                    nc.tensor.matmul(pb[:], E_T[:], vB[:],
                                     start=True, stop=True)
                    nc.vector.scalar_tensor_tensor(
                        vB[:], pb[:], 1.0,
                        F_rec[:, BL * s:BL * s + BL],
                        op0=OP.mult, op1=OP.mult)
                # late sprinkles that didn't fit in n_steps
                for k in sorted(sprinkle):
                    if k >= n_steps:
                        for item in sprinkle[k]:
                            if item[0] == "F":
                                f_tile(item[1], head=False)
                            else:
                                lab_tile(item[1])
                # beta_m, then Z_part[b] = sum_j alpha_m[j,b] * beta_m[j,b]
                pz2 = psm.tile([L, BL], f32, tag="pf")
                nc.tensor.matmul(pz2[:], E_T[:], vB[:], start=True, stop=True)
                tmpz = const.tile([L, BL], f32, tag="tmpz")
                nc.vector.scalar_tensor_tensor(
                    tmpz[:], pz2[:], 1.0, alphaF[:], op0=OP.mult, op1=OP.mult)

            if not done["ln"]:
                emit_ln()
            if not done["gh"]:
                emit_gh()

            # ---- finalization ----
            with tc.tile_pool(name="psf", bufs=1,
                              space=bass.MemorySpace.PSUM) as psf:
                pz = psf.tile([1, BL], f32, tag="pz")
                nc.tensor.matmul(pz[:], ones128[:L, :], tmpz[:],
                                 start=True, stop=True)
                pd = psf.tile([1, nt], f32, tag="pd")
                nc.tensor.matmul(pd[:], ones128[:], LS[:], start=True, stop=True)
                pg = psf.tile([1, 2], f32, tag="pg")
                nc.tensor.matmul(pg[:], ones128[:L, :], gh[:],
                                 start=True, stop=True)

                lnz = const.tile([1, BL], f32, tag="lnz")
                nc.scalar.activation(lnz[:], pz[:], AF.Ln, bias=zero1[:])

                td = const.tile([1, BL], f32, tag="td")
                nc.vector.tensor_reduce(
                    td[:], pd[:].rearrange("p (b c) -> p b c", c=tpb),
                    axis=AX.X, op=OP.add)

                v1 = const.tile([1, BL], f32, tag="v1")
                nc.vector.tensor_add(v1[:], lnz[:], td[:])
                t0 = const.tile([1, 1], f32, tag="t0")
                nc.vector.tensor_reduce(t0[:], v1[:], axis=AX.X, op=OP.add)
                t1 = const.tile([1, 1], f32, tag="t1")
                nc.vector.tensor_sub(t1[:], t0[:], pg[:, 0:1])
                tot = const.tile([1, 1], f32, tag="tot")
                nc.vector.tensor_sub(tot[:], t1[:], pg[:, 1:2])
                nc.sync.dma_start(out.ap(), tot[:])

    nc.compile()
    return nc


@functools.lru_cache(maxsize=1)
def _built():
    return build_crf_bass(T)


def kernel(inputs: np.ndarray, labels: np.ndarray, trans: np.ndarray) -> np.ndarray:
    from concourse.bass_utils import run_bass_kernel_spmd

    nc = _built()
    inputs = np.ascontiguousarray(inputs, dtype=np.float32)
    labels = np.ascontiguousarray(labels, dtype=np.float32)
    trans = np.ascontiguousarray(trans, dtype=np.float32)
    in_maps = [
        {
            "inputs": inputs[c * BL:(c + 1) * BL],
            "labels": labels[c * BL:(c + 1) * BL],
            "trans": trans,
        }
        for c in range(NCORES)
    ]
    res = run_bass_kernel_spmd(nc, in_maps, core_ids=list(range(NCORES)))
    total = np.float64(0.0)
    for r in res.results:
        total += np.float64(r["out"][0, 0])
    return np.array(total, dtype=np.float32)


# revision 27
# speedup vs baseline: 1.0055x; 1.0055x over previous
"""CRF negative-log-likelihood loss kernel for Trainium2 (Bass/Tile).

Problem: B=32, T=512, L=64 linear-chain CRF loss
    loss = sum_b [ -path_score(b) + logZ(b) ]

Algorithm (per core; data-parallel over batch, 4 rows/core):
  logZ via the linear-space scaled forward recurrence, run CONCURRENTLY
  from both ends (forward-backward identity) to halve the serial span:
      F_t[j] = exp(h_t[j]) / S_t          (softmax of emissions, sum=1)
      alpha_t = diag(F_t) E^T alpha_{t-1},   E = exp(trans)
      beta_{s-1} = E (F_s ⊙ beta_s)
      Z = sum_j alpha_m[j] beta_m[j];  logZ = ln Z + sum_t ln S_t
  The F normalization keeps both states bounded (empirically [1,10]) so
  bf16/fp32 stay in range with no max-subtraction (inputs ~ N(0,1)).
  Each chain step: one bf16 PE matmul (stationary E / E^T) + one DVE
  scalar_tensor_tensor (PSUM * F -> SBUF bf16). State is (64 part, 4 b).

  path scores via PSUM-accumulated cross-products (summed over b - the
  loss sums b anyway):
      h_total = trace(inp_flat^T @ lab_flat)
      g_total = <trans, C>,  C = lab_flat[:-1]^T @ lab_flat[1:]
  (boundary tiles use 127 rows so no cross-batch transitions leak in).
  These 32 (128,64,64) matmuls + input prep (exp on ScalarE, transpose
  on PE) are sprinkled between chain steps to hide in engine slack.

Each core emits its partial loss scalar; the host sums the 8 partials
(the scalar all-reduce of the sharding hint).
"""

import functools

import numpy as np

B, T, L = 32, 512, 64
NCORES = 8
BL = B // NCORES  # 4 batch rows per core
P = 128


def build_crf_bass(t_len: int = T):
    """Build the per-core Bass/Tile program. Returns the compiled Bass object."""
    import concourse.bass as bass
    import concourse.bacc as bacc
    import concourse.mybir as mybir
    from concourse import masks
    from concourse import tile

    f32 = mybir.dt.float32
    bf16 = mybir.dt.bfloat16
    AX = mybir.AxisListType
    OP = mybir.AluOpType
    AF = mybir.ActivationFunctionType

    nt = BL * t_len // P  # input tiles
    tpb = t_len // P      # tiles per batch row
    assert t_len % P == 0 and t_len >= 2 * P

    nc = bacc.Bacc("TRN2", target_bir_lowering=False, debug=False,
                   enable_asserts=False)

    inputs = nc.dram_tensor("inputs", [BL, t_len, L], f32, kind="ExternalInput")
    labels = nc.dram_tensor("labels", [BL, t_len, L], f32, kind="ExternalInput")
    trans = nc.dram_tensor("trans", [L, L], f32, kind="ExternalInput")
    out = nc.dram_tensor("out", [1, 1], f32, kind="ExternalOutput")

    inp_flat = inputs.ap().rearrange("b t l -> (b t) l")  # (BL*T, L)
    lab_flat = labels.ap().rearrange("b t l -> (b t) l")

    # Asymmetric meeting point: the forward chain starts earlier (its F
    # tiles prep first), so it runs PRO extra prologue steps while the
    # backward group is still prepping.  fwd does m steps, bwd T-2-m.
    PRO = 8
    n_steps = (t_len - 2 - PRO) // 2   # interleaved fwd+bwd waves
    m = PRO + n_steps                  # meeting point: alpha_m (x) beta_m

    with tile.TileContext(nc) as tc:
        with (
            tc.tile_pool(name="const", bufs=1) as const,
            tc.tile_pool(name="stream", bufs=3) as stream,
            tc.tile_pool(name="pst", bufs=2, space=bass.MemorySpace.PSUM) as pst,
            tc.tile_pool(name="psc", bufs=1, space=bass.MemorySpace.PSUM) as psc,
        ):
            ident = const.tile([P, P], f32, tag="ident")
            masks.make_identity(nc, ident[:])
            zeros128 = const.tile([P, 1], f32, tag="z128")
            nc.vector.memset(zeros128[:], 0.0)
            zero1 = const.tile([1, 1], f32, tag="z1")
            nc.vector.memset(zero1[:], 0.0)
            ones128 = const.tile([P, 1], f32, tag="o128")
            nc.vector.memset(ones128[:], 1.0)

            S = const.tile([P, nt], f32, tag="S")      # sum_j exp(h)
            R = const.tile([P, nt], f32, tag="R")      # 1/S
            LS = const.tile([P, nt], f32, tag="LS")    # ln S

            F_rec = const.tile([L, t_len * BL], f32, tag="F_rec")  # F[j, t*4+b]
            tr_sb = const.tile([L, L], f32, tag="tr")
            E_sb = const.tile([L, L], bf16, tag="E")    # exp(trans), lhsT fwd
            E_T = const.tile([L, L], bf16, tag="ET")    # exp(trans)^T, lhsT bwd

            # warm the ACT exp table while the first DMAs are in flight
            tbl = const.tile([1, 1], f32, tag="tbl")
            nc.scalar.activation(tbl[:], zero1[:], AF.Exp, bias=zero1[:])

            nc.sync.dma_start(tr_sb[:], trans.ap())
            nc.scalar.activation(E_sb[:], tr_sb[:], AF.Exp,
                                 bias=zeros128[:L, :])
            psE = pst.tile([L, L], f32, tag="tp")
            nc.tensor.transpose(psE[:], tr_sb[:], ident[:L, :L])
            nc.scalar.activation(E_T[:], psE[:], AF.Exp, bias=zeros128[:L, :])

            # PSUM accumulators for the path scores
            C_ps = psc.tile([L, L], f32, tag="C")       # transition counts
            H_ps = psc.tile([L, L], f32, tag="H")       # inp^T @ lab

            F_v = F_rec[:].rearrange("j (t b) -> j t b", b=BL)

            def f_tile_body(c, head, inp_ap):
                """exp/normalize/transpose one (128,64) input tile into F_rec."""
                b, th = divmod(c, tpb)
                fe_c = stream.tile([P, L], f32, tag="fe")
                nc.scalar.activation(fe_c[:], inp_ap, AF.Exp,
                                     bias=zeros128[:, :1],
                                     accum_out=S[:, c:c + 1])
                nc.vector.reciprocal(R[:, c:c + 1], S[:, c:c + 1])
                fn_c = stream.tile([P, L], f32, tag="fn")
                if head:
                    nc.vector.tensor_scalar_mul(fn_c[:], fe_c[:], R[:, c:c + 1])
                else:
                    nc.scalar.mul(fn_c[:], fe_c[:], R[:, c:c + 1])
                psF = pst.tile([L, P], f32, tag="tp")
                nc.tensor.transpose(psF[:], fn_c[:], ident[:])
                if head:
                    nc.vector.tensor_copy(F_v[:, th * P:(th + 1) * P, b], psF[:])
                else:
                    nc.scalar.copy(F_v[:, th * P:(th + 1) * P, b], psF[:])

            def f_tile(c, head):
                inp_c = stream.tile([P, L], f32, tag="inp")
                nc.sync.dma_start(inp_c[:], inp_flat[c * P:(c + 1) * P, :])
                f_tile_body(c, head, inp_c[:])

            cc_state = {"c": 0, "h": 0}

            def lab_tile(c):
                """Accumulate this tile into the C and H cross-products."""
                kk = P - 1 if (c % tpb == tpb - 1) else P
                inp_c = stream.tile([P, L], f32, tag="inp2")
                nc.sync.dma_start(inp_c[:], inp_flat[c * P:(c + 1) * P, :])
                lab_c = stream.tile([P, L], f32, tag="lab")
                nc.sync.dma_start(lab_c[:], lab_flat[c * P:(c + 1) * P, :])
                lsh_c = stream.tile([P, L], f32, tag="lsh")
                nc.sync.dma_start(lsh_c[:kk, :],
                                  lab_flat[c * P + 1:c * P + 1 + kk, :])
                nc.tensor.matmul(C_ps[:], lab_c[:kk, :], lsh_c[:kk, :],
                                 start=(cc_state["c"] == 0),
                                 stop=(cc_state["c"] == nt - 1),
                                 skip_group_check=True)
                cc_state["c"] += 1
                nc.tensor.matmul(H_ps[:], inp_c[:], lab_c[:],
                                 start=(cc_state["h"] == 0),
                                 stop=(cc_state["h"] == nt - 1),
                                 skip_group_check=True)
                cc_state["h"] += 1

            # F tiles needed to start both chains: th=0 (fwd) and th=tpb-1 (bwd)
            # loaded as one wide DMA per th group, processed with batched
            # (128, BL*64) ops so the head critical path stays short.
            head = [b * tpb for b in range(BL)]
            if tpb > 1:
                head += [b * tpb + tpb - 1 for b in range(BL)]
            inp_4d = inputs.ap().rearrange("b (th p) l -> p th b l", p=P)

            def head_group(th_g):
                big = stream.tile([P, BL * L], f32, tag=f"big{th_g}")
                nc.sync.dma_start(
                    big[:].rearrange("p (b l) -> p b l", b=BL),
                    inp_4d[:, th_g, :, :])
                fe_g = stream.tile([P, BL * L], f32, tag=f"feg{th_g}")
                nc.scalar.activation(fe_g[:], big[:], AF.Exp,
                                     bias=zeros128[:, :1])
                s_g = S[:, th_g::tpb]            # (128, BL) strided
                nc.vector.tensor_reduce(
                    s_g, fe_g[:].rearrange("p (b l) -> p b l", b=BL),
                    axis=AX.X, op=OP.add)
                nc.vector.reciprocal(R[:, th_g::tpb], s_g)
                fn_g = stream.tile([P, BL * L], f32, tag=f"fng{th_g}")
                nc.vector.tensor_mul(
                    fn_g[:].rearrange("p (b l) -> p b l", b=BL),
                    fe_g[:].rearrange("p (b l) -> p b l", b=BL),
                    R[:, th_g::tpb].broadcast_to([P, BL, L]))
                for b in range(BL):
                    fn_c = fn_g[:, b * L:(b + 1) * L]
                    psF = pst.tile([L, P], f32, tag="tp")
                    nc.tensor.transpose(psF[:], fn_c, ident[:])
                    nc.vector.tensor_copy(
                        F_v[:, th_g * P:(th_g + 1) * P, b], psF[:])

            mid = [c for c in range(nt) if c not in head]

            # sprinkle schedule: remaining F tiles early, label tiles after
            sprinkle = {}
            for i, c in enumerate(mid):
                sprinkle.setdefault(6 + 10 * i, []).append(("F", c))
            lab_start = 6 + 10 * len(mid) + 6
            for i in range(nt):
                sprinkle.setdefault(lab_start + 9 * i, []).append(("L", i))

            alphaF = const.tile([L, BL], bf16, tag="alphaF")
            vB = const.tile([L, BL], bf16, tag="vB")
            head_group(0)
            nc.vector.tensor_copy(alphaF[:], F_rec[:, 0:BL])

            ln_wave = 6 + 10 * len(mid) + 4
            gh_wave = lab_start + 9 * nt + 6
            gh = const.tile([L, 2], f32, tag="gh")

            def emit_ln():
                nc.scalar.activation(LS[:], S[:], AF.Ln, bias=zeros128[:, :1])

            def emit_gh():
                # path scores from the accumulated cross-products:
                #   g_total = <trans, C>;  h_total = trace(H) = <ident, H>
                gtmp = stream.tile([L, L], f32, tag="gtmp")
                nc.vector.scalar_tensor_tensor(gtmp[:], C_ps[:], 1.0, tr_sb[:],
                                               op0=OP.mult, op1=OP.mult)
                nc.vector.tensor_reduce(gh[:, 0:1], gtmp[:], axis=AX.X,
                                        op=OP.add)
                htmp = stream.tile([L, L], f32, tag="htmp")
                nc.vector.scalar_tensor_tensor(htmp[:], H_ps[:], 1.0,
                                               ident[:L, :L],
                                               op0=OP.mult, op1=OP.mult)
                nc.vector.tensor_reduce(gh[:, 1:2], htmp[:], axis=AX.X,
                                        op=OP.add)

            done = {"ln": False, "gh": False}
            with tc.tile_pool(name="psm", bufs=2,
                              space=bass.MemorySpace.PSUM) as psm:

                def fwd_step(t):
                    pf = psm.tile([L, BL], f32, tag="pf")
                    nc.tensor.matmul(pf[:], E_sb[:], alphaF[:],
                                     start=True, stop=True)
                    nc.vector.scalar_tensor_tensor(
                        alphaF[:], pf[:], 1.0,
                        F_rec[:, BL * t:BL * t + BL],
                        op0=OP.mult, op1=OP.mult)

                def bwd_step(s):
                    pb = psm.tile([L, BL], f32, tag="pb")
                    nc.tensor.matmul(pb[:], E_T[:], vB[:],
                                     start=True, stop=True)
                    nc.vector.scalar_tensor_tensor(
                        vB[:], pb[:], 1.0,
                        F_rec[:, BL * s:BL * s + BL],
                        op0=OP.mult, op1=OP.mult)

                # fwd-only prologue while the backward F group preps
                for pk in range(PRO):
                    if pk == 0 and tpb > 1:
                        head_group(tpb - 1)
                    if pk == 2:
                        nc.vector.tensor_copy(
                            vB[:], F_rec[:, (t_len - 1) * BL:t_len * BL])
                    fwd_step(pk + 1)
                if tpb == 1 and PRO == 0:
                    pass

                for k in range(n_steps):
                    for item in sprinkle.get(k, []):
                        if item[0] == "F":
                            f_tile(item[1], head=False)
                        else:
                            lab_tile(item[1])
                    if k == ln_wave:
                        emit_ln()
                        done["ln"] = True
                    if k == gh_wave:
                        emit_gh()
                        done["gh"] = True
                    fwd_step(PRO + 1 + k)
                    bwd_step(t_len - 2 - k)
                # late sprinkles that didn't fit in n_steps
                for k in sorted(sprinkle):
                    if k >= n_steps:
                        for item in sprinkle[k]:
                            if item[0] == "F":
                                f_tile(item[1], head=False)
                            else:
                                lab_tile(item[1])
                # beta_m, then Z_part[b] = sum_j alpha_m[j,b] * beta_m[j,b]
                pz2 = psm.tile([L, BL], f32, tag="pf")
                nc.tensor.matmul(pz2[:], E_T[:], vB[:], start=True, stop=True)
                tmpz = const.tile([L, BL], f32, tag="tmpz")
                nc.vector.scalar_tensor_tensor(
                    tmpz[:], pz2[:], 1.0, alphaF[:], op0=OP.mult, op1=OP.mult)

            if not done["ln"]:
                emit_ln()
            if not done["gh"]:
                emit_gh()

            # ---- finalization ----
            with tc.tile_pool(name="psf", bufs=1,
                              space=bass.MemorySpace.PSUM) as psf:
                pz = psf.tile([1, BL], f32, tag="pz")
                nc.tensor.matmul(pz[:], ones128[:L, :], tmpz[:],
                                 start=True, stop=True)
                pd = psf.tile([1, nt], f32, tag="pd")
                nc.tensor.matmul(pd[:], ones128[:], LS[:], start=True, stop=True)
                pg = psf.tile([1, 2], f32, tag="pg")
                nc.tensor.matmul(pg[:], ones128[:L, :], gh[:],
                                 start=True, stop=True)

                lnz = const.tile([1, BL], f32, tag="lnz")
                nc.scalar.activation(lnz[:], pz[:], AF.Ln, bias=zero1[:])

                td = const.tile([1, BL], f32, tag="td")
                nc.vector.tensor_reduce(
                    td[:], pd[:].rearrange("p (b c) -> p b c", c=tpb),
                    axis=AX.X, op=OP.add)

                v1 = const.tile([1, BL], f32, tag="v1")
                nc.vector.tensor_add(v1[:], lnz[:], td[:])
                t0 = const.tile([1, 1], f32, tag="t0")
                nc.vector.tensor_reduce(t0[:], v1[:], axis=AX.X, op=OP.add)
                t1 = const.tile([1, 1], f32, tag="t1")
                nc.vector.tensor_sub(t1[:], t0[:], pg[:, 0:1])
                tot = const.tile([1, 1], f32, tag="tot")
                nc.vector.tensor_sub(tot[:], t1[:], pg[:, 1:2])
                nc.sync.dma_start(out.ap(), tot[:])

    nc.compile()
    return nc


@functools.lru_cache(maxsize=1)
def _built():
    return build_crf_bass(T)


def kernel(inputs: np.ndarray, labels: np.ndarray, trans: np.ndarray) -> np.ndarray:
    from concourse.bass_utils import run_bass_kernel_spmd

    nc = _built()
    inputs = np.ascontiguousarray(inputs, dtype=np.float32)
    labels = np.ascontiguousarray(labels, dtype=np.float32)
    trans = np.ascontiguousarray(trans, dtype=np.float32)
    in_maps = [
        {
            "inputs": inputs[c * BL:(c + 1) * BL],
            "labels": labels[c * BL:(c + 1) * BL],
            "trans": trans,
        }
        for c in range(NCORES)
    ]
    res = run_bass_kernel_spmd(nc, in_maps, core_ids=list(range(NCORES)))
    total = np.float64(0.0)
    for r in res.results:
        total += np.float64(r["out"][0, 0])
    return np.array(total, dtype=np.float32)


# revision 33
# speedup vs baseline: 1.0080x; 1.0025x over previous
"""CRF negative-log-likelihood loss kernel for Trainium2 (Bass/Tile).

Problem: B=32, T=512, L=64 linear-chain CRF loss
    loss = sum_b [ -path_score(b) + logZ(b) ]

Algorithm (per core; data-parallel over batch, 4 rows/core):
  logZ via the linear-space scaled forward recurrence, run CONCURRENTLY
  from both ends (forward-backward identity) to halve the serial span:
      F_t[j] = exp(h_t[j]) / S_t          (softmax of emissions, sum=1)
      alpha_t = diag(F_t) E^T alpha_{t-1},   E = exp(trans)
      beta_{s-1} = E (F_s ⊙ beta_s)
      Z = sum_j alpha_m[j] beta_m[j];  logZ = ln Z + sum_t ln S_t
  The F normalization keeps both states bounded (empirically [1,10]) so
  bf16/fp32 stay in range with no max-subtraction (inputs ~ N(0,1)).
  Each chain step: one bf16 PE matmul (stationary E / E^T) + one DVE
  scalar_tensor_tensor (PSUM * F -> SBUF bf16). State is (64 part, 4 b).

  path scores via PSUM-accumulated cross-products (summed over b - the
  loss sums b anyway):
      h_total = trace(inp_flat^T @ lab_flat)
      g_total = <trans, C>,  C = lab_flat[:-1]^T @ lab_flat[1:]
  (boundary tiles use 127 rows so no cross-batch transitions leak in).
  These 32 (128,64,64) matmuls + input prep (exp on ScalarE, transpose
  on PE) are sprinkled between chain steps to hide in engine slack.

Each core emits its partial loss scalar; the host sums the 8 partials
(the scalar all-reduce of the sharding hint).
"""

import functools

import numpy as np

B, T, L = 32, 512, 64
NCORES = 8
BL = B // NCORES  # 4 batch rows per core
P = 128


def build_crf_bass(t_len: int = T):
    """Build the per-core Bass/Tile program. Returns the compiled Bass object."""
    import concourse.bass as bass
    import concourse.bacc as bacc
    import concourse.mybir as mybir
    from concourse import masks
    from concourse import tile

    f32 = mybir.dt.float32
    bf16 = mybir.dt.bfloat16
    AX = mybir.AxisListType
    OP = mybir.AluOpType
    AF = mybir.ActivationFunctionType

    nt = BL * t_len // P  # input tiles
    tpb = t_len // P      # tiles per batch row
    assert t_len % P == 0 and t_len >= 2 * P

    nc = bacc.Bacc("TRN2", target_bir_lowering=False, debug=False,
                   enable_asserts=False)

    inputs = nc.dram_tensor("inputs", [BL, t_len, L], f32, kind="ExternalInput")
    labels = nc.dram_tensor("labels", [BL, t_len, L], f32, kind="ExternalInput")
    trans = nc.dram_tensor("trans", [L, L], f32, kind="ExternalInput")
    out = nc.dram_tensor("out", [1, 1], f32, kind="ExternalOutput")

    inp_flat = inputs.ap().rearrange("b t l -> (b t) l")  # (BL*T, L)
    lab_flat = labels.ap().rearrange("b t l -> (b t) l")

    # Asymmetric meeting point: the forward chain starts earlier (its F
    # tiles prep first), so it runs PRO extra prologue steps while the
    # backward group is still prepping.  fwd does m steps, bwd T-2-m.
    PRO = 8
    n_steps = (t_len - 2 - PRO) // 2   # interleaved fwd+bwd waves
    m = PRO + n_steps                  # meeting point: alpha_m (x) beta_m

    with tile.TileContext(nc) as tc:
        with (
            tc.tile_pool(name="const", bufs=1) as const,
            tc.tile_pool(name="stream", bufs=3) as stream,
            tc.tile_pool(name="pst", bufs=2, space=bass.MemorySpace.PSUM) as pst,
            tc.tile_pool(name="psc", bufs=1, space=bass.MemorySpace.PSUM) as psc,
        ):
            ident = const.tile([P, P], f32, tag="ident")
            masks.make_identity(nc, ident[:])
            zeros128 = const.tile([P, 1], f32, tag="z128")
            nc.vector.memset(zeros128[:], 0.0)
            zero1 = const.tile([1, 1], f32, tag="z1")
            nc.vector.memset(zero1[:], 0.0)
            ones128 = const.tile([P, 1], f32, tag="o128")
            nc.vector.memset(ones128[:], 1.0)

            S = const.tile([P, nt], f32, tag="S")      # sum_j exp(h)
            R = const.tile([P, nt], f32, tag="R")      # 1/S
            LS = const.tile([P, nt], f32, tag="LS")    # ln S

            F_rec = const.tile([L, t_len * BL], f32, tag="F_rec")  # F[j, t*4+b]
            tr_sb = const.tile([L, L], f32, tag="tr")
            E_sb = const.tile([L, L], bf16, tag="E")    # exp(trans), lhsT fwd
            E_T = const.tile([L, L], bf16, tag="ET")    # exp(trans)^T, lhsT bwd

            # warm the ACT exp table while the first DMAs are in flight
            tbl = const.tile([1, 1], f32, tag="tbl")
            nc.scalar.activation(tbl[:], zero1[:], AF.Exp, bias=zero1[:])

            nc.sync.dma_start(tr_sb[:], trans.ap())
            nc.scalar.activation(E_sb[:], tr_sb[:], AF.Exp,
                                 bias=zeros128[:L, :])
            psE = pst.tile([L, L], f32, tag="tp")
            nc.tensor.transpose(psE[:], tr_sb[:], ident[:L, :L])
            nc.scalar.activation(E_T[:], psE[:], AF.Exp, bias=zeros128[:L, :])

            # PSUM accumulators for the path scores
            C_ps = psc.tile([L, L], f32, tag="C")       # transition counts
            H_ps = psc.tile([L, L], f32, tag="H")       # inp^T @ lab

            F_v = F_rec[:].rearrange("j (t b) -> j t b", b=BL)

            def f_tile_body(c, head, inp_ap):
                """exp/normalize/transpose one (128,64) input tile into F_rec."""
                b, th = divmod(c, tpb)
                fe_c = stream.tile([P, L], f32, tag="fe")
                nc.scalar.activation(fe_c[:], inp_ap, AF.Exp,
                                     bias=zeros128[:, :1],
                                     accum_out=S[:, c:c + 1])
                nc.vector.reciprocal(R[:, c:c + 1], S[:, c:c + 1])
                fn_c = stream.tile([P, L], f32, tag="fn")
                if head:
                    nc.vector.tensor_scalar_mul(fn_c[:], fe_c[:], R[:, c:c + 1])
                else:
                    nc.scalar.mul(fn_c[:], fe_c[:], R[:, c:c + 1])
                psF = pst.tile([L, P], f32, tag="tp")
                nc.tensor.transpose(psF[:], fn_c[:], ident[:])
                if head:
                    nc.vector.tensor_copy(F_v[:, th * P:(th + 1) * P, b], psF[:])
                else:
                    nc.scalar.copy(F_v[:, th * P:(th + 1) * P, b], psF[:])

            def f_tile(c, head):
                inp_c = stream.tile([P, L], f32, tag="inp")
                nc.sync.dma_start(inp_c[:], inp_flat[c * P:(c + 1) * P, :])
                f_tile_body(c, head, inp_c[:])

            cc_state = {"c": 0, "h": 0}
            lab_tiles = {}

            def lab_dma(c):
                """Stage the three input tiles for one C/H accumulation."""
                inp_c = stream.tile([P, L], f32, tag="inp2")
                nc.sync.dma_start(inp_c[:], inp_flat[c * P:(c + 1) * P, :])
                lab_c = stream.tile([P, L], f32, tag="lab")
                nc.sync.dma_start(lab_c[:], lab_flat[c * P:(c + 1) * P, :])
                kk = P - 1 if (c % tpb == tpb - 1) else P
                lsh_c = stream.tile([P, L], f32, tag="lsh")
                nc.sync.dma_start(lsh_c[:kk, :],
                                  lab_flat[c * P + 1:c * P + 1 + kk, :])
                lab_tiles[c] = (inp_c, lab_c, lsh_c, kk)

            def lab_mm_c(c):
                _, lab_c, lsh_c, kk = lab_tiles[c]
                nc.tensor.matmul(C_ps[:], lab_c[:kk, :], lsh_c[:kk, :],
                                 start=(cc_state["c"] == 0),
                                 stop=(cc_state["c"] == nt - 1),
                                 skip_group_check=True)
                cc_state["c"] += 1

            def lab_mm_h(c):
                inp_c, lab_c, _, _ = lab_tiles[c]
                nc.tensor.matmul(H_ps[:], inp_c[:], lab_c[:],
                                 start=(cc_state["h"] == 0),
                                 stop=(cc_state["h"] == nt - 1),
                                 skip_group_check=True)
                cc_state["h"] += 1
                del lab_tiles[c]

            # F tiles needed to start both chains: th=0 (fwd) and th=tpb-1 (bwd)
            # loaded as one wide DMA per th group, processed with batched
            # (128, BL*64) ops so the head critical path stays short.
            head = [b * tpb for b in range(BL)]
            if tpb > 1:
                head += [b * tpb + tpb - 1 for b in range(BL)]
            inp_4d = inputs.ap().rearrange("b (th p) l -> p th b l", p=P)

            def head_group(th_g):
                big = stream.tile([P, BL * L], f32, tag=f"big{th_g}")
                nc.sync.dma_start(
                    big[:].rearrange("p (b l) -> p b l", b=BL),
                    inp_4d[:, th_g, :, :])
                fe_g = stream.tile([P, BL * L], f32, tag=f"feg{th_g}")
                nc.scalar.activation(fe_g[:], big[:], AF.Exp,
                                     bias=zeros128[:, :1])
                s_g = S[:, th_g::tpb]            # (128, BL) strided
                nc.vector.tensor_reduce(
                    s_g, fe_g[:].rearrange("p (b l) -> p b l", b=BL),
                    axis=AX.X, op=OP.add)
                nc.vector.reciprocal(R[:, th_g::tpb], s_g)
                fn_g = stream.tile([P, BL * L], f32, tag=f"fng{th_g}")
                nc.vector.tensor_mul(
                    fn_g[:].rearrange("p (b l) -> p b l", b=BL),
                    fe_g[:].rearrange("p (b l) -> p b l", b=BL),
                    R[:, th_g::tpb].broadcast_to([P, BL, L]))
                for b in range(BL):
                    fn_c = fn_g[:, b * L:(b + 1) * L]
                    psF = pst.tile([L, P], f32, tag="tp")
                    nc.tensor.transpose(psF[:], fn_c, ident[:])
                    nc.vector.tensor_copy(
                        F_v[:, th_g * P:(th_g + 1) * P, b], psF[:])

            mid = [c for c in range(nt) if c not in head]

            # sprinkle schedule: remaining F tiles early, then label tiles
            # with their DMAs and the two accumulation matmuls staggered so
            # no single wave absorbs a large engine burst.
            sprinkle = {}
            for i, c in enumerate(mid):
                sprinkle.setdefault(6 + 14 * i, []).append(("F", c))
            lab_start = 6 + 14 * len(mid) - 6
            for i in range(nt):
                sprinkle.setdefault(lab_start + 8 * i, []).append(("LD", i))
                sprinkle.setdefault(lab_start + 8 * i + 4, []).append(("LC", i))
                sprinkle.setdefault(lab_start + 8 * i + 6, []).append(("LH", i))

            alphaF = const.tile([L, BL], bf16, tag="alphaF")
            vB = const.tile([L, BL], bf16, tag="vB")
            head_group(0)
            nc.vector.tensor_copy(alphaF[:], F_rec[:, 0:BL])

            ln_wave = 6 + 14 * len(mid) + 2
            gh_wave = lab_start + 8 * (nt - 1) + 6 + 4
            fin_wave = gh_wave + 4
            gh = const.tile([L, 2], f32, tag="gh")

            def emit_ln():
                nc.scalar.activation(LS[:], S[:], AF.Ln, bias=zeros128[:, :1])

            def emit_gh():
                # path scores from the accumulated cross-products:
                #   g_total = <trans, C>;  h_total = trace(H) = <ident, H>
                gtmp = stream.tile([L, L], f32, tag="gtmp")
                nc.vector.scalar_tensor_tensor(gtmp[:], C_ps[:], 1.0, tr_sb[:],
                                               op0=OP.mult, op1=OP.mult)
                nc.vector.tensor_reduce(gh[:, 0:1], gtmp[:], axis=AX.X,
                                        op=OP.add)
                htmp = stream.tile([L, L], f32, tag="htmp")
                nc.vector.scalar_tensor_tensor(htmp[:], H_ps[:], 1.0,
                                               ident[:L, :L],
                                               op0=OP.mult, op1=OP.mult)
                nc.vector.tensor_reduce(gh[:, 1:2], htmp[:], axis=AX.X,
                                        op=OP.add)

            done = {"ln": False, "gh": False, "fin": False}
            EMIT = {"F": lambda c: f_tile(c, head=False), "LD": lab_dma,
                    "LC": lab_mm_c, "LH": lab_mm_h}
            pd = const.tile([1, nt], f32, tag="pd_s")
            pg = const.tile([1, 2], f32, tag="pg_s")
            with tc.tile_pool(name="psm", bufs=2,
                              space=bass.MemorySpace.PSUM) as psm:

                def emit_fin():
                    # column sums of ln S and of the path-score pair, done
                    # inside the chain window via spare psm slots
                    ppd = psm.tile([1, nt], f32, tag="pf")
                    nc.tensor.matmul(ppd[:], ones128[:], LS[:],
                                     start=True, stop=True)
                    nc.vector.tensor_copy(pd[:], ppd[:])
                    ppg = psm.tile([1, 2], f32, tag="pb")
                    nc.tensor.matmul(ppg[:], ones128[:L, :], gh[:],
                                     start=True, stop=True)
                    nc.vector.tensor_copy(pg[:], ppg[:])

                def fwd_step(t):
                    pf = psm.tile([L, BL], f32, tag="pf")
                    nc.tensor.matmul(pf[:], E_sb[:], alphaF[:],
                                     start=True, stop=True)
                    nc.vector.scalar_tensor_tensor(
                        alphaF[:], pf[:], 1.0,
                        F_rec[:, BL * t:BL * t + BL],
                        op0=OP.mult, op1=OP.mult)

                def bwd_step(s):
                    pb = psm.tile([L, BL], f32, tag="pb")
                    nc.tensor.matmul(pb[:], E_T[:], vB[:],
                                     start=True, stop=True)
                    nc.vector.scalar_tensor_tensor(
                        vB[:], pb[:], 1.0,
                        F_rec[:, BL * s:BL * s + BL],
                        op0=OP.mult, op1=OP.mult)

                # fwd-only prologue while the backward F group preps
                for pk in range(PRO):
                    if pk == 0 and tpb > 1:
                        head_group(tpb - 1)
                    if pk == 2:
                        nc.vector.tensor_copy(
                            vB[:], F_rec[:, (t_len - 1) * BL:t_len * BL])
                    fwd_step(pk + 1)
                if tpb == 1 and PRO == 0:
                    pass

                for k in range(n_steps):
                    for item in sprinkle.get(k, []):
                        EMIT[item[0]](item[1])
                    if k == ln_wave:
                        emit_ln()
                        done["ln"] = True
                    if k == gh_wave:
                        emit_gh()
                        done["gh"] = True
                    if k == fin_wave:
                        emit_fin()
                        done["fin"] = True
                    fwd_step(PRO + 1 + k)
                    bwd_step(t_len - 2 - k)
                # late sprinkles that didn't fit in n_steps
                for k in sorted(sprinkle):
                    if k >= n_steps:
                        for item in sprinkle[k]:
                            EMIT[item[0]](item[1])
                if not done["ln"]:
                    emit_ln()
                    done["ln"] = True
                if not done["gh"]:
                    emit_gh()
                    done["gh"] = True
                if not done["fin"]:
                    emit_fin()
                    done["fin"] = True
                # beta_m, then Z_part[b] = sum_j alpha_m[j,b] * beta_m[j,b]
                pz2 = psm.tile([L, BL], f32, tag="pf")
                nc.tensor.matmul(pz2[:], E_T[:], vB[:], start=True, stop=True)
                tmpz = const.tile([L, BL], f32, tag="tmpz")
                nc.vector.scalar_tensor_tensor(
                    tmpz[:], pz2[:], 1.0, alphaF[:], op0=OP.mult, op1=OP.mult)

            # ---- finalization ----
            with tc.tile_pool(name="psf", bufs=1,
                              space=bass.MemorySpace.PSUM) as psf:
                pz = psf.tile([1, BL], f32, tag="pz")
                nc.tensor.matmul(pz[:], ones128[:L, :], tmpz[:],
                                 start=True, stop=True)

                lnz = const.tile([1, BL], f32, tag="lnz")
                nc.scalar.activation(lnz[:], pz[:], AF.Ln, bias=zero1[:])

                td = const.tile([1, BL], f32, tag="td")
                nc.vector.tensor_reduce(
                    td[:], pd[:].rearrange("p (b c) -> p b c", c=tpb),
                    axis=AX.X, op=OP.add)

                v1 = const.tile([1, BL], f32, tag="v1")
                nc.vector.tensor_add(v1[:], lnz[:], td[:])
                t0 = const.tile([1, 1], f32, tag="t0")
                nc.vector.tensor_reduce(t0[:], v1[:], axis=AX.X, op=OP.add)
                t1 = const.tile([1, 1], f32, tag="t1")
                nc.vector.tensor_sub(t1[:], t0[:], pg[:, 0:1])
                tot = const.tile([1, 1], f32, tag="tot")
                nc.vector.tensor_sub(tot[:], t1[:], pg[:, 1:2])
                nc.sync.dma_start(out.ap(), tot[:])

    nc.compile()
    return nc


@functools.lru_cache(maxsize=1)
def _built():
    return build_crf_bass(T)


def kernel(inputs: np.ndarray, labels: np.ndarray, trans: np.ndarray) -> np.ndarray:
    from concourse.bass_utils import run_bass_kernel_spmd

    nc = _built()
    inputs = np.ascontiguousarray(inputs, dtype=np.float32)
    labels = np.ascontiguousarray(labels, dtype=np.float32)
    trans = np.ascontiguousarray(trans, dtype=np.float32)
    in_maps = [
        {
            "inputs": inputs[c * BL:(c + 1) * BL],
            "labels": labels[c * BL:(c + 1) * BL],
            "trans": trans,
        }
        for c in range(NCORES)
    ]
    res = run_bass_kernel_spmd(nc, in_maps, core_ids=list(range(NCORES)))
    total = np.float64(0.0)
    for r in res.results:
        total += np.float64(r["out"][0, 0])
    return np.array(total, dtype=np.float32)
